# revision 2
# baseline (speedup 1.0000x reference)
"""BiLSTM (B=128, T=256, H=512, L=2) Trainium2 Bass kernel — cross-core stages.

8 cores = 2 directions x 4 role stages, full batch B=128 per core:
  rank % 4 == 0: G0  xproj0 = x @ Wx0 + b0        (dir = rank // 4)
  rank % 4 == 1: S0  layer-0 LSTM scan            (consumes xproj0, emits h0^T)
  rank % 4 == 2: G1  xproj1 = h0 @ Wx1 + b1
  rank % 4 == 3: S1  layer-1 LSTM scan -> out
Stages run sequentially with whole-tensor pairwise AllGather handoffs.
Gate columns are host-reordered to [i, f, o, g] so sigmoid/tanh each cover
one contiguous span.
"""

import numpy as np

import concourse.bacc as bacc
import concourse.mybir as mybir
import concourse.tile as tile
from concourse import bass_utils
from concourse.masks import make_identity

F32 = mybir.dt.float32
BF16 = mybir.dt.bfloat16
AF = mybir.ActivationFunctionType
OP = mybir.AluOpType

B, T, H, L = 128, 256, 512, 2
G = 4 * H          # 2048
KT = H // 128      # 4
NT = G // 512      # 4
NCORES = 8

RG_A = [[0, 1], [2, 3], [4, 5], [6, 7]]   # G0 -> S0 (pairs 0-1, 4-5 real)
RG_B = [[1, 2], [0, 3], [5, 6], [4, 7]]   # S0 -> G1 (pairs 1-2, 5-6 real)
RG_C = [[0, 1], [2, 3], [4, 5], [6, 7]]   # G1 -> S1 (pairs 2-3, 6-7 real)


def build_bilstm(T=T, reps=1):
    nc = bacc.Bacc("TRN2", target_bir_lowering=False, debug=False, num_devices=NCORES)

    # role-dependent content, same tensor names on every core
    w_in = nc.dram_tensor("w", [128, KT, G], BF16, kind="ExternalInput").ap()
    bias_in = nc.dram_tensor("bias", [128, G], F32, kind="ExternalInput").ap()
    xt_in = nc.dram_tensor("xt", [128, KT, T, 128], BF16, kind="ExternalInput").ap()
    out = nc.dram_tensor("out", [128, T, H], BF16, kind="ExternalOutput").ap()

    with tile.TileContext(nc) as tc:
        with (
            tc.tile_pool(name="const", bufs=1) as const,
            tc.tile_pool(name="sb", bufs=2) as sb,
            tc.tile_pool(name="sb1", bufs=1) as sb1,
            tc.tile_pool(name="ps", bufs=1, space="PSUM") as psp,
            tc.tile_pool(name="ps2", bufs=2, space="PSUM") as psp2,
            tc.tile_pool(name="dram", bufs=1, space="DRAM") as dram,
        ):
            ident = const.tile([128, 128], F32)
            make_identity(nc, ident)
            identr = const.tile([128, 128], BF16)
            nc.vector.tensor_copy(identr[:], ident[:])
            w_sb = const.tile([128, KT, G], BF16)
            bias_sb = const.tile([128, G], F32)
            nc.sync.dma_start(w_sb[:], w_in[:])
            nc.sync.dma_start(bias_sb[:], bias_in[:])

            rank = nc.partition_id()

            def gemm_all(stat_of, cc_dst):
                """T m-tiles: xo = stat^T @ w + bias -> cc_dst[:, t, :]."""
                for t4 in range(T // 4):
                    xo4 = sb.tile([128, 4, G], BF16, tag="xo4")
                    for j in range(4):
                        t = t4 * 4 + j
                        stat = stat_of(t)  # callable kt -> lhsT [128, 128]
                        pg = psp.tile([128, G], F32, tag="pg")
                        for kt in range(KT):
                            for n in range(NT):
                                nc.tensor.matmul(
                                    pg[:, n * 512:(n + 1) * 512],
                                    stat(kt),
                                    w_sb[:, kt, n * 512:(n + 1) * 512],
                                    start=(kt == 0),
                                    stop=(kt == KT - 1),
                                )
                        nc.vector.tensor_tensor(
                            xo4[:, j, :], pg[:], bias_sb[:], op=OP.add
                        )
                    nc.sync.dma_start(
                        cc_dst[:, t4 * 4:(t4 + 1) * 4, :], xo4[:]
                    )

            def scan_all(src_cc_out, is_s0, ccB_i):
                """T LSTM steps; src xproj = src_cc_out[0] [128, T, G]."""
                prev_hT = None
                prev_c = None
                for t in range(T):
                    first = t == 0
                    if t % 2 == 0:
                        xp2 = sb1.tile([128, 2, G], BF16, tag="xp2")
                        nc.sync.dma_start(
                            xp2[:], src_cc_out[0][:, t:t + 2, :]
                        )
                    if t % 4 == 0:
                        h4 = sb1.tile([128, 4, H], BF16, tag="h4")
                        hT4 = sb1.tile([128, 4, KT, 128], BF16, tag="hT4")
                    xp = xp2[:, t % 2, :]

                    if first:
                        ga_src = xp
                    else:
                        pg = psp.tile([128, G], F32, tag="pg")
                        for kt in range(KT):
                            for n in range(NT):
                                nc.tensor.matmul(
                                    pg[:, n * 512:(n + 1) * 512],
                                    prev_hT[:, kt, :],
                                    w_sb[:, kt, n * 512:(n + 1) * 512],
                                    start=(kt == 0),
                                    stop=(kt == KT - 1),
                                )
                        ga = sb1.tile([128, G], BF16, tag="ga")
                        nc.vector.tensor_tensor(ga[:], pg[:], xp, op=OP.add)
                        ga_src = ga[:]

                    gact = sb1.tile([128, G], BF16, tag="gact")
                    nc.scalar.activation(
                        gact[:, 0:1536], ga_src[:, 0:1536], AF.Sigmoid
                    )
                    nc.scalar.activation(
                        gact[:, 1536:2048], ga_src[:, 1536:2048], AF.Tanh
                    )
                    # columns: i 0:512 | f 512:1024 | o 1024:1536 | g 1536:2048
                    c_new = sb.tile([128, H], F32, tag="c_st")
                    if first:
                        nc.vector.tensor_tensor(
                            c_new[:], gact[:, 0:512], gact[:, 1536:2048],
                            op=OP.mult,
                        )
                    else:
                        m1 = sb1.tile([128, H], F32, tag="m1")
                        nc.vector.tensor_tensor(
                            m1[:], gact[:, 0:512], gact[:, 1536:2048],
                            op=OP.mult,
                        )
                        m2 = sb1.tile([128, H], F32, tag="m2")
                        nc.vector.tensor_tensor(
                            m2[:], gact[:, 512:1024], prev_c[:], op=OP.mult
                        )
                        nc.vector.tensor_tensor(
                            c_new[:], m1[:], m2[:], op=OP.add
                        )
                    tch = sb1.tile([128, H], F32, tag="tch")
                    nc.scalar.activation(tch[:], c_new[:], AF.Tanh)
                    nc.vector.tensor_tensor(
                        h4[:, t % 4, :], gact[:, 1024:1536], tch[:], op=OP.mult
                    )
                    ptp = psp2.tile([128, KT, 128], BF16, tag="ptp")
                    for kt in range(KT):
                        nc.tensor.transpose(
                            ptp[:, kt, :],
                            h4[:, t % 4, kt * 128:(kt + 1) * 128],
                            identr[:],
                        )
                    nc.vector.tensor_copy(hT4[:, t % 4, :, :], ptp[:])
                    prev_hT = hT4[:, t % 4, :, :]
                    prev_c = c_new

                    if t % 4 == 3:
                        if is_s0:
                            nc.sync.dma_start(
                                ccB_i[:, t - 3:t + 1, :, :], hT4[:]
                            )
                        else:
                            nc.sync.dma_start(
                                out[:, t - 3:t + 1, :], h4[:]
                            )

            for _rep in range(reps):
                ccA_i = dram.tile([128, T, G], BF16, tag="ccA_i")
                ccA_o = dram.tile([2, 128, T, G], BF16, tag="ccA_o")
                ccB_i = dram.tile([128, T, KT, 128], BF16, tag="ccB_i")
                ccB_o = dram.tile([2, 128, T, KT, 128], BF16, tag="ccB_o")
                ccC_i = dram.tile([128, T, G], BF16, tag="ccC_i")
                ccC_o = dram.tile([2, 128, T, G], BF16, tag="ccC_o")

                with tc.If(rank % 4 == 0):
                    cur = {}

                    def stat_of_x(t):
                        if t % 4 == 0:
                            xt4 = sb.tile([128, KT, 4, 128], BF16, tag="xt4")
                            nc.sync.dma_start(xt4[:], xt_in[:, :, t:t + 4, :])
                            cur["xt4"] = xt4
                        return lambda kt, j=t % 4: cur["xt4"][:, kt, j, :]

                    gemm_all(stat_of_x, ccA_i)

                nc.gpsimd.collective_compute(
                    "AllGather", OP.bypass, replica_groups=RG_A,
                    ins=[ccA_i[:].opt()], outs=[ccA_o[:].opt()],
                )

                with tc.If(rank % 4 == 1):
                    scan_all(ccA_o, True, ccB_i)

                nc.gpsimd.collective_compute(
                    "AllGather", OP.bypass, replica_groups=RG_B,
                    ins=[ccB_i[:].opt()], outs=[ccB_o[:].opt()],
                )

                with tc.If(rank % 4 == 2):
                    cur1 = {}

                    def stat_of_h(t):
                        if t % 8 == 0:
                            hT8 = sb.tile([128, 8, KT, 128], BF16, tag="hT8")
                            nc.sync.dma_start(
                                hT8[:], ccB_o[0][:, t:t + 8, :, :]
                            )
                            cur1["hT8"] = hT8
                        return lambda kt, j=t % 8: cur1["hT8"][:, j, kt, :]

                    gemm_all(stat_of_h, ccC_i)

                nc.gpsimd.collective_compute(
                    "AllGather", OP.bypass, replica_groups=RG_C,
                    ins=[ccC_i[:].opt()], outs=[ccC_o[:].opt()],
                )

                with tc.If(rank % 4 == 3):
                    scan_all(ccC_o, False, None)

    nc.compile()
    return nc


_NC_CACHE = {}


def _get_nc():
    if "nc" not in _NC_CACHE:
        _NC_CACHE["nc"] = build_bilstm()
    return _NC_CACHE["nc"]


def _perm_cols(a):
    """gate columns [i f g o] -> [i f o g] along last axis (size G)."""
    return np.concatenate(
        [a[..., 0:512], a[..., 512:1024], a[..., 1536:2048], a[..., 1024:1536]],
        axis=-1,
    )


def _pack_w(w):
    """[H, G] f32 -> [128, KT, G] bf16 (partition-major k-tiles)."""
    import ml_dtypes

    w = _perm_cols(w).reshape(KT, 128, G).transpose(1, 0, 2)
    return np.ascontiguousarray(w.astype(ml_dtypes.bfloat16))


def _pack_bias(b):
    return np.ascontiguousarray(
        np.broadcast_to(_perm_cols(b), (128, G)), dtype=np.float32
    )


def _pack_xt(x):
    """[B, T, H] f32 -> [128, KT, T, 128] bf16  (h_p, kt, t, b)."""
    import ml_dtypes

    xt = x.transpose(2, 1, 0)          # [H, T, B]
    xt = xt.reshape(KT, 128, T, B).transpose(1, 0, 2, 3)
    return np.ascontiguousarray(xt.astype(ml_dtypes.bfloat16))


def _shard_inputs(x, Wx, Wh, b):
    import ml_dtypes

    zw = np.zeros((128, KT, G), ml_dtypes.bfloat16)
    zb = np.zeros((128, G), np.float32)
    zx = np.zeros((128, KT, T, 128), ml_dtypes.bfloat16)
    in_maps = []
    for r in range(NCORES):
        d = r // 4
        stage = r % 4
        xd = x if d == 0 else np.ascontiguousarray(x[:, ::-1, :])
        if stage == 0:
            m = {"w": _pack_w(Wx[0, d]), "bias": _pack_bias(b[0, d]),
                 "xt": _pack_xt(xd)}
        elif stage == 1:
            m = {"w": _pack_w(Wh[0, d]), "bias": zb, "xt": zx}
        elif stage == 2:
            m = {"w": _pack_w(Wx[1, d]), "bias": _pack_bias(b[1, d]), "xt": zx}
        else:
            m = {"w": _pack_w(Wh[1, d]), "bias": zb, "xt": zx}
        in_maps.append(m)
    return in_maps


def _assemble(results):
    fwd = np.asarray(results[3]["out"], dtype=np.float32)
    bwd = np.asarray(results[7]["out"], dtype=np.float32)[:, ::-1, :]
    return np.concatenate([fwd, bwd], axis=-1)


def run_kernel(x, Wx, Wh, b, trace=False):
    nc = _get_nc()
    in_maps = _shard_inputs(
        np.asarray(x), np.asarray(Wx), np.asarray(Wh), np.asarray(b)
    )
    res = bass_utils.run_bass_kernel_spmd(
        nc, in_maps, core_ids=list(range(NCORES)), trace=trace
    )
    return _assemble(res.results), res


def kernel(x, Wx, Wh, b):
    out, _ = run_kernel(x, Wx, Wh, b)
    return out


# revision 3
# speedup vs baseline: 2.6044x; 2.6044x over previous
"""BiLSTM (B=128, T=256, H=512, L=2) Trainium2 Bass kernel, v4.

Sharding: 8 cores = 2 directions x 4 batch-quarters (B_loc=32), data-parallel.
Each core runs 4 sequential phases: xproj0 GEMM -> layer-0 scan -> xproj1 GEMM
-> layer-1 scan. Host pre-flips time for backward cores and re-assembles.

Instruction-count-oriented choices:
 - float32r matmuls (self-loading: no separate Ldweights instruction)
 - gate columns host-reordered to [i, f, o, g]: one sigmoid over 1536 cols,
   one tanh over 512
 - h^T for the next step's matmuls produced by a DRAM round-trip with a
   transposing access pattern (2 DMAs) instead of 4 PE transposes + copy
 - batched DMAs (4 steps of xproj per load, 2 GEMM m-tiles per store)
"""

import numpy as np

import concourse.bacc as bacc
import concourse.mybir as mybir
import concourse.tile as tile
from concourse import bass_utils

F32 = mybir.dt.float32
F32R = mybir.dt.float32r
BF16 = mybir.dt.bfloat16
AF = mybir.ActivationFunctionType
OP = mybir.AluOpType

B_FULL, T_FULL, H, L = 128, 256, 512, 2
G = 4 * H          # 2048
KT = H // 128      # 4
NT = G // 512      # 4
NCORES = 8
B = B_FULL // 4    # 32 per core


def build_bilstm(T=T_FULL, reps=1):
    assert T % 8 == 0
    nc = bacc.Bacc("TRN2", target_bir_lowering=False, debug=False)

    xt_in = nc.dram_tensor("xt", [128, KT, T // 4, 128], F32R,
                           kind="ExternalInput").ap()
    wx0_in = nc.dram_tensor("wx0", [128, KT, G], F32R, kind="ExternalInput").ap()
    wh0_in = nc.dram_tensor("wh0", [128, KT, G], F32R, kind="ExternalInput").ap()
    wx1_in = nc.dram_tensor("wx1", [128, KT, G], F32R, kind="ExternalInput").ap()
    wh1_in = nc.dram_tensor("wh1", [128, KT, G], F32R, kind="ExternalInput").ap()
    b0_in = nc.dram_tensor("b0", [128, G], F32, kind="ExternalInput").ap()
    b1_in = nc.dram_tensor("b1", [128, G], F32, kind="ExternalInput").ap()
    # out[t, kt, p, b] = h1[b, t, kt*128+p]
    out = nc.dram_tensor("out", [T, KT, 128, B], F32R, kind="ExternalOutput").ap()

    with tile.TileContext(nc) as tc:
        with (
            tc.tile_pool(name="sb", bufs=2) as sb,
            tc.tile_pool(name="sb1", bufs=1) as sb1,
            tc.tile_pool(name="ps", bufs=1, space="PSUM") as psp,
            tc.tile_pool(name="dram", bufs=1, space="DRAM") as dram,
        ):
            def load_w(w_ap, b_ap):
                wsb = sb1.tile([128, KT, G], F32R, tag="wsb")
                nc.sync.dma_start(wsb[:], w_ap[:])
                if b_ap is None:
                    return wsb, None
                bsb = sb1.tile([128, G], F32, tag="bsb")
                nc.sync.dma_start(bsb[:], b_ap[:])
                return wsb, bsb

            def gemm(stat_of, wsb, bsb, xp_dst):
                """T//4 m-tiles (rows = 4 t-major timesteps x 32 batch)."""
                for mt in range(T // 4):
                    stat = stat_of(mt)     # kt -> lhsT [128, 128]
                    pg = psp.tile([128, G], F32, tag="pg")
                    for kt in range(KT):
                        for n in range(NT):
                            nc.tensor.matmul(
                                pg[:, n * 512:(n + 1) * 512],
                                stat(kt),
                                wsb[:, kt, n * 512:(n + 1) * 512],
                                start=(kt == 0),
                                stop=(kt == KT - 1),
                            )
                    if mt % 2 == 0:
                        xo2 = sb.tile([128, 2, G], BF16, tag="xo2")
                    nc.vector.tensor_tensor(
                        xo2[:, mt % 2, :], pg[:], bsb[:], op=OP.add
                    )
                    if mt % 2 == 1:
                        nc.sync.dma_start(
                            xp_dst[mt * 4 - 4:mt * 4 + 4, :, :].rearrange(
                                "(m t) b g -> (t b) m g", m=2
                            ),
                            xo2[:],
                        )

            def scan(wsb, xp_src, hbufT):
                """T LSTM steps reading xproj [T, 32, G], writing hbufT
                [128, KT, T, 32] (transposed h history)."""
                prev_c = None
                for t in range(T):
                    first = t == 0
                    if t % 4 == 0:
                        xp4 = sb1.tile([32, 4, G], BF16, tag="xp4")
                        nc.sync.dma_start(
                            xp4[:],
                            xp_src[t:t + 4, :, :].rearrange("t b g -> b t g"),
                        )
                    xp = xp4[:, t % 4, :]

                    if first:
                        ga_src = xp
                    else:
                        pg = psp.tile([B, G], F32, tag="pg")
                        for kt in range(KT):
                            for n in range(NT):
                                nc.tensor.matmul(
                                    pg[:, n * 512:(n + 1) * 512],
                                    hT[:, kt, :],
                                    wsb[:, kt, n * 512:(n + 1) * 512],
                                    start=(kt == 0),
                                    stop=(kt == KT - 1),
                                )
                        ga = sb1.tile([32, G], BF16, tag="ga")
                        nc.vector.tensor_tensor(ga[:], pg[:], xp, op=OP.add)
                        ga_src = ga[:]

                    gact = sb1.tile([32, G], BF16, tag="gact")
                    nc.scalar.activation(
                        gact[:, 0:1536], ga_src[:, 0:1536], AF.Sigmoid
                    )
                    nc.scalar.activation(
                        gact[:, 1536:2048], ga_src[:, 1536:2048], AF.Tanh
                    )
                    # cols: i 0:512 | f 512:1024 | o 1024:1536 | g 1536:2048
                    c_new = sb.tile([32, H], F32, tag="c_st")
                    if first:
                        nc.vector.tensor_tensor(
                            c_new[:], gact[:, 0:512], gact[:, 1536:2048],
                            op=OP.mult,
                        )
                    else:
                        m1 = sb1.tile([32, H], F32, tag="m1")
                        nc.vector.tensor_tensor(
                            m1[:], gact[:, 0:512], gact[:, 1536:2048],
                            op=OP.mult,
                        )
                        m2 = sb1.tile([32, H], F32, tag="m2")
                        nc.vector.tensor_tensor(
                            m2[:], gact[:, 512:1024], prev_c[:], op=OP.mult
                        )
                        nc.vector.tensor_tensor(
                            c_new[:], m1[:], m2[:], op=OP.add
                        )
                    tch = sb1.tile([32, H], F32, tag="tch")
                    nc.scalar.activation(tch[:], c_new[:], AF.Tanh)
                    h = sb1.tile([32, KT, 128], F32R, tag="h")
                    nc.vector.tensor_tensor(
                        h[:].rearrange("b kt p -> b (kt p)"),
                        gact[:, 1024:1536], tch[:], op=OP.mult,
                    )
                    # h -> DRAM, then load back transposed as hT
                    nc.sync.dma_start(
                        hbufT[t, :, :, :].rearrange("kt p b -> b kt p"), h[:]
                    )
                    hT = sb.tile([128, KT, B], F32R, tag="hT")
                    nc.sync.dma_start(
                        hT[:],
                        hbufT[t, :, :, :].rearrange("kt p b -> p kt b"),
                    )
                    prev_c = c_new

            for _rep in range(reps):
                xproj0 = dram.tile([T, B, G], BF16, tag="xproj0")
                xproj1 = dram.tile([T, B, G], BF16, tag="xproj1")
                hbufT0 = dram.tile([T, KT, 128, B], F32R, tag="hbufT0")

                # P1: xproj0 = x @ wx0 + b0
                wsb, bsb = load_w(wx0_in, b0_in)
                cur = {}

                def stat_x(mt):
                    if mt % 2 == 0:
                        xt2 = sb.tile([128, KT, 2, 128], F32R, tag="xt2")
                        nc.sync.dma_start(
                            xt2[:], xt_in[:, :, mt:mt + 2, :]
                        )
                        cur["xt2"] = xt2
                    return lambda kt, m=mt % 2: cur["xt2"][:, kt, m, :]

                gemm(stat_x, wsb, bsb, xproj0[:])

                # P2: layer-0 scan
                wsb, _ = load_w(wh0_in, None)
                scan(wsb, xproj0[:], hbufT0[:])

                # P3: xproj1 = h0 @ wx1 + b1
                wsb, bsb = load_w(wx1_in, b1_in)
                cur1 = {}

                def stat_h(mt):
                    if mt % 2 == 0:
                        hT8 = sb.tile([128, KT, 8, B], F32R, tag="hT8")
                        for kt in range(KT):
                            nc.sync.dma_start(
                                hT8[:, kt, :, :],
                                hbufT0[mt * 4:mt * 4 + 8, kt, :, :].rearrange(
                                    "t p b -> p t b"
                                ),
                            )
                        cur1["hT8"] = hT8
                    return lambda kt, m=mt % 2: cur1["hT8"][
                        :, kt, m * 4:(m + 1) * 4, :
                    ].rearrange("p t b -> p (t b)")

                gemm(stat_h, wsb, bsb, xproj1[:])

                # P4: layer-1 scan -> out
                wsb, _ = load_w(wh1_in, None)
                scan(wsb, xproj1[:], out)

    nc.compile()
    return nc


_NC_CACHE = {}


def _get_nc(T=T_FULL):
    if T not in _NC_CACHE:
        _NC_CACHE[T] = build_bilstm(T=T)
    return _NC_CACHE[T]


def _perm_cols(a):
    """gate columns [i f g o] -> [i f o g] along last axis (size G)."""
    return np.concatenate(
        [a[..., 0:512], a[..., 512:1024], a[..., 1536:2048], a[..., 1024:1536]],
        axis=-1,
    )


def _pack_w(w):
    w = _perm_cols(w).reshape(KT, 128, G).transpose(1, 0, 2)
    return np.ascontiguousarray(w, dtype=np.float32)


def _pack_bias(b):
    return np.ascontiguousarray(
        np.broadcast_to(_perm_cols(b), (128, G)), dtype=np.float32
    )


def _pack_xt(x, T):
    """[B, T, H] -> [128, KT, T//4, 128] : [p, kt, mt, (4t x 32b) t-major]."""
    xt = x.transpose(2, 1, 0)                       # [H, T, B]
    xt = xt.reshape(KT, 128, T // 4, 4, B)
    xt = xt.transpose(1, 0, 2, 3, 4).reshape(128, KT, T // 4, 128)
    return np.ascontiguousarray(xt, dtype=np.float32)


def _shard_inputs(x, Wx, Wh, b):
    T = x.shape[1]
    in_maps = []
    packed = {}
    for d in range(2):
        packed[d] = {
            "wx0": _pack_w(Wx[0, d]), "wh0": _pack_w(Wh[0, d]),
            "wx1": _pack_w(Wx[1, d]), "wh1": _pack_w(Wh[1, d]),
            "b0": _pack_bias(b[0, d]), "b1": _pack_bias(b[1, d]),
        }
    for r in range(NCORES):
        d, q = r // 4, r % 4
        xc = x[q * B:(q + 1) * B]
        if d == 1:
            xc = xc[:, ::-1, :]
        m = dict(packed[d])
        m["xt"] = _pack_xt(xc, T)
        in_maps.append(m)
    return in_maps


def _assemble(results, T=T_FULL):
    full = np.empty((B_FULL, T, 2 * H), dtype=np.float32)
    for r in range(NCORES):
        d, q = r // 4, r % 4
        o = np.asarray(results[r]["out"], dtype=np.float32)  # [T, KT, 128, B]
        o = o.transpose(3, 0, 1, 2).reshape(B, T, H)
        if d == 1:
            o = o[:, ::-1, :]
        full[q * B:(q + 1) * B, :, d * H:(d + 1) * H] = o
    return full


def run_kernel(x, Wx, Wh, b, trace=False):
    nc = _get_nc()
    in_maps = _shard_inputs(
        np.asarray(x), np.asarray(Wx), np.asarray(Wh), np.asarray(b)
    )
    res = bass_utils.run_bass_kernel_spmd(
        nc, in_maps, core_ids=list(range(NCORES)), trace=trace
    )
    return _assemble(res.results), res


def kernel(x, Wx, Wh, b):
    out, _ = run_kernel(x, Wx, Wh, b)
    return out
